# revision 1
# baseline (speedup 1.0000x reference)
"""Trainium2 Bass kernel for the biaffine pairwise relation scorer.

Model (per reference):
  h_src = (hidden @ W_src.T + b_src).reshape(B, L, R, H)
  h_tgt = (hidden @ W_tgt.T + b_tgt).reshape(B, L, R, H)
  rel[b, r, s, t]   = sum_h tanh(h_src[b,s,r,h] + h_tgt[b,t,r,h]) * w_out[h]
  mask[b, tt, l]    = sum_h tanh((hidden @ dense_W.T + dense_b)[b,l,tt,h]) * clf_W[0,h] + clf_b

Sharding: 8 cores <-> (b, r) in {0,1} x {0..3}.  Each core computes the full
L x L pairwise block for its (b, r) entirely on-chip (the (B,L,L,R,H)
intermediate never touches HBM).  The tiny dense head is split by h-range
(192 per core within each batch); host sums the partials.

Per-core engine plan:
  PE : projections (h_srcT/h_tgtT, layout [h, token]) + w_out contraction
       (M=1 accumulating matmuls over 6 h-chunks)
  DVE: pairwise broadcast-add  pair[h, s, t] = srcT[h,s] + tgtT[h,t]
  ACT: tanh over the pair tiles (the throughput floor: ~12.6M elem/core)
"""

import sys

if "/opt/trn_rl_repo" not in sys.path:
    sys.path.insert(0, "/opt/trn_rl_repo")

import numpy as np

B, L, H, R, T = 2, 128, 768, 4, 3
P = 128
KC = H // P            # 6 h-chunks of 128
SB = 32                # source-positions per pairwise block
NSB = L // SB          # 4 blocks
NSUB = 8               # 512-wide reduce sub-blocks per pairwise block
HRANGE = H // 4        # dense-head h columns per core (4 cores per batch)
DM = 96                # dense-head M-chunk (HRANGE*T rows = 576 = 6 x 96)
NDM = (HRANGE * T) // DM

_cache = {}


def _build():
    import concourse.bacc as bacc
    import concourse.tile as tile
    from concourse import mybir

    f32 = mybir.dt.float32
    AF = mybir.ActivationFunctionType
    OP = mybir.AluOpType

    nc = bacc.Bacc("TRN2", target_bir_lowering=False, debug=False)

    # ---- DRAM I/O (per-core views; host pre-transposes/pre-tiles) ----
    hidT_d = nc.dram_tensor("hidT", (KC, P, L), f32, kind="ExternalInput")
    wsrcT_d = nc.dram_tensor("wsrcT", (KC, KC, P, P), f32, kind="ExternalInput")
    wtgtT_d = nc.dram_tensor("wtgtT", (KC, KC, P, P), f32, kind="ExternalInput")
    bsrc_d = nc.dram_tensor("bsrc", (1, H), f32, kind="ExternalInput")
    btgt_d = nc.dram_tensor("btgt", (1, H), f32, kind="ExternalInput")
    wout_d = nc.dram_tensor("wout", (P, KC), f32, kind="ExternalInput")
    dwT_d = nc.dram_tensor("dwT", (KC, NDM, P, DM), f32, kind="ExternalInput")
    db_d = nc.dram_tensor("db", (DM, NDM), f32, kind="ExternalInput")
    clf_d = nc.dram_tensor("clf", (DM, NDM // T), f32, kind="ExternalInput")

    rel_d = nc.dram_tensor("rel", (NSB * NSUB, 512), f32, kind="ExternalOutput")
    hsrc_d = nc.dram_tensor("hsrc", (KC, P, L), f32, kind="ExternalOutput")
    htgt_d = nc.dram_tensor("htgt", (KC, P, L), f32, kind="ExternalOutput")
    maskp_d = nc.dram_tensor("maskp", (T, L), f32, kind="ExternalOutput")

    with tile.TileContext(nc) as tc:
        with (
            tc.tile_pool(name="consts", bufs=1) as consts,
            tc.tile_pool(name="wpool", bufs=8) as wpool,
            tc.tile_pool(name="projsb", bufs=1) as projsb,
            tc.tile_pool(name="pairp", bufs=7) as pairp,
            tc.tile_pool(name="evacp", bufs=4) as evacp,
            tc.tile_pool(name="mmps", bufs=2, space="PSUM") as mmps,
            tc.tile_pool(name="rowps", bufs=3, space="PSUM") as rowps,
        ):
            # ---------------- constants ----------------
            ones = consts.tile([1, P], f32, name="ones", tag="ones")
            nc.vector.memset(ones[:], 1.0)

            hidT = consts.tile([P, KC * P], f32, name="hidT_sb", tag="hidT_sb")
            for kc in range(KC):
                nc.sync.dma_start(hidT[:, kc * P:(kc + 1) * P], hidT_d[kc])

            bsrc = consts.tile([1, H], f32, name="bsrc_sb", tag="bsrc_sb")
            nc.sync.dma_start(bsrc[:], bsrc_d[:])
            btgt = consts.tile([1, H], f32, name="btgt_sb", tag="btgt_sb")
            nc.sync.dma_start(btgt[:], btgt_d[:])
            wout = consts.tile([P, KC], f32, name="wout_sb", tag="wout_sb")
            nc.sync.dma_start(wout[:], wout_d[:])
            db = consts.tile([DM, NDM], f32, name="db_sb", tag="db_sb")
            nc.sync.dma_start(db[:], db_d[:])
            clf = consts.tile([DM, NDM // T], f32, name="clf_sb", tag="clf_sb")
            nc.sync.dma_start(clf[:], clf_d[:])

            srcT = projsb.tile([P, KC * P], f32, name="srcT", tag="srcT")
            tgtT = projsb.tile([P, KC * P], f32, name="tgtT", tag="tgtT")

            # ---------------- projections (PE) ----------------
            # h_xT[m-chunk] = sum_kc W_xT[kc,m].T @ hidT[kc]  + b_x (rank-1)
            for m in range(KC):
                for w_d, b_sb, outT, out_d, nm in (
                    (wsrcT_d, bsrc, srcT, hsrc_d, "s"),
                    (wtgtT_d, btgt, tgtT, htgt_d, "t"),
                ):
                    ps = mmps.tile([P, P], f32, name=f"ps_{nm}{m}", tag="proj")
                    for kc in range(KC):
                        wt = wpool.tile([P, P], f32, name=f"w_{nm}{m}_{kc}", tag="wt")
                        nc.sync.dma_start(wt[:], w_d[kc, m])
                        nc.tensor.matmul(
                            ps[:], wt[:], hidT[:, kc * P:(kc + 1) * P],
                            start=(kc == 0), stop=False,
                        )
                    nc.tensor.matmul(
                        ps[:], b_sb[:, m * P:(m + 1) * P], ones[:],
                        start=False, stop=True,
                    )
                    nc.scalar.copy(outT[:, m * P:(m + 1) * P], ps[:])
                    nc.sync.dma_start(out_d[m], outT[:, m * P:(m + 1) * P])

            # ---------------- dense head (tiny) ----------------
            zt = projsb.tile([DM, NDM * P], f32, name="zt", tag="zt")
            for m in range(NDM):
                psd = mmps.tile([DM, P], f32, name=f"psd{m}", tag="dense")
                for kc in range(KC):
                    wt = wpool.tile([P, DM], f32, name=f"wd{m}_{kc}", tag="wtd")
                    nc.sync.dma_start(wt[:], dwT_d[kc, m])
                    nc.tensor.matmul(
                        psd[:], wt[:], hidT[:, kc * P:(kc + 1) * P],
                        start=(kc == 0), stop=(kc == KC - 1),
                    )
                nc.scalar.activation(
                    zt[:, m * P:(m + 1) * P], psd[:], AF.Tanh,
                    bias=db[:, m:m + 1],
                )
            for tt in range(T):
                pm = rowps.tile([1, P], f32, name=f"pm{tt}", tag="row")
                for j in range(NDM // T):
                    m = tt * (NDM // T) + j
                    nc.tensor.matmul(
                        pm[:], clf[:, j:j + 1], zt[:, m * P:(m + 1) * P],
                        start=(j == 0), stop=(j == NDM // T - 1),
                    )
                ev = evacp.tile([1, P], f32, name=f"mev{tt}", tag="ev")
                nc.vector.tensor_copy(ev[:], pm[:])
                nc.sync.dma_start(maskp_d[tt], ev[:])

            # ---------------- pairwise (DVE add -> ACT tanh -> PE reduce) ----
            for sb in range(NSB):
                s0 = sb * SB
                ptiles = []
                for kc in range(KC):
                    pt = pairp.tile([P, SB, P], f32, name=f"pair{sb}_{kc}", tag="pair")
                    nc.vector.tensor_tensor(
                        pt[:],
                        srcT[:, kc * P + s0: kc * P + s0 + SB][:, :, None]
                        .to_broadcast((P, SB, P)),
                        tgtT[:, None, kc * P:(kc + 1) * P]
                        .to_broadcast((P, SB, P)),
                        op=OP.add,
                    )
                    nc.scalar.activation(pt[:], pt[:], AF.Tanh)
                    ptiles.append(pt)
                for sub in range(NSUB):
                    sl = sub * 4          # 4 source positions per 512-col slab
                    pr = rowps.tile([1, 512], f32, name=f"pr{sb}_{sub}", tag="row")
                    for kc in range(KC):
                        nc.tensor.matmul(
                            pr[:], wout[:, kc:kc + 1],
                            ptiles[kc][:, sl:sl + 4, :],
                            start=(kc == 0), stop=(kc == KC - 1),
                        )
                    ev = evacp.tile([1, 512], f32, name=f"rev{sb}_{sub}", tag="ev")
                    if (sb * NSUB + sub) % 2 == 0:
                        nc.vector.tensor_copy(ev[:], pr[:])
                    else:
                        nc.scalar.copy(ev[:], pr[:])
                    nc.sync.dma_start(rel_d[sb * NSUB + sub], ev[:])

    nc.compile()
    return nc


def _in_maps(inputs):
    hidden = np.asarray(inputs["hidden_state"], np.float32)
    W_src = np.asarray(inputs["W_src"], np.float32)
    b_src = np.asarray(inputs["b_src"], np.float32)
    W_tgt = np.asarray(inputs["W_tgt"], np.float32)
    b_tgt = np.asarray(inputs["b_tgt"], np.float32)
    w_out = np.asarray(inputs["w_out"], np.float32)
    dense_W = np.asarray(inputs["dense_W"], np.float32)
    dense_b = np.asarray(inputs["dense_b"], np.float32)
    clf_W = np.asarray(inputs["clf_W"], np.float32)

    def tile_wT(w_block, mdim):
        # [rows, H] weight block -> lhsT chunks (kc, m, k, mcol)
        wT = np.ascontiguousarray(w_block.T)          # [H(k), rows(m)]
        nm = w_block.shape[0] // mdim
        return np.ascontiguousarray(
            wT.reshape(KC, P, nm, mdim).transpose(0, 2, 1, 3)
        )

    wout_t = np.ascontiguousarray(w_out.reshape(KC, P).T)  # [128, 6]

    maps = []
    for c in range(8):
        b, r = c // 4, c % 4
        hr0 = (c % 4) * HRANGE
        rows = np.concatenate(
            [np.arange(tt * H + hr0, tt * H + hr0 + HRANGE) for tt in range(T)]
        )
        maps.append({
            "hidT": np.ascontiguousarray(hidden[b].T).reshape(KC, P, L),
            "wsrcT": tile_wT(W_src[r * H:(r + 1) * H], P),
            "wtgtT": tile_wT(W_tgt[r * H:(r + 1) * H], P),
            "bsrc": np.ascontiguousarray(b_src[r * H:(r + 1) * H]).reshape(1, H),
            "btgt": np.ascontiguousarray(b_tgt[r * H:(r + 1) * H]).reshape(1, H),
            "wout": wout_t,
            "dwT": tile_wT(dense_W[rows], DM),
            "db": np.ascontiguousarray(dense_b[rows].reshape(NDM, DM).T),
            "clf": np.ascontiguousarray(clf_W[0, hr0:hr0 + HRANGE].reshape(NDM // T, DM).T),
        })
    return maps


def _assemble(results, inputs):
    clf_b = np.asarray(inputs["clf_b"], np.float32)
    rel = np.empty((B, R, L, L), np.float32)
    h_src = np.empty((B, L, R, H), np.float32)
    h_tgt = np.empty((B, L, R, H), np.float32)
    mask = np.zeros((B, T, L), np.float32)
    for c in range(8):
        b, r = c // 4, c % 4
        out = results[c]
        rel[b, r] = out["rel"].reshape(L, L)
        h_src[b, :, r, :] = out["hsrc"].transpose(2, 0, 1).reshape(L, H)
        h_tgt[b, :, r, :] = out["htgt"].transpose(2, 0, 1).reshape(L, H)
        mask[b] += out["maskp"]
    mask += clf_b[0]
    return rel, mask, h_src, h_tgt


def _run(inputs, trace=False):
    from concourse import bass_utils

    if "nc" not in _cache:
        _cache["nc"] = _build()
    res = bass_utils.run_bass_kernel_spmd(
        _cache["nc"], _in_maps(inputs), core_ids=list(range(8)), trace=trace,
    )
    return _assemble(res.results, inputs), res


def kernel(**inputs):
    out, _ = _run(inputs, trace=False)
    return out


# revision 6
# speedup vs baseline: 1.3233x; 1.3233x over previous
"""Trainium2 Bass kernel for the biaffine pairwise relation scorer.

Model (per reference):
  h_src = (hidden @ W_src.T + b_src).reshape(B, L, R, H)
  h_tgt = (hidden @ W_tgt.T + b_tgt).reshape(B, L, R, H)
  rel[b, r, s, t]   = sum_h tanh(h_src[b,s,r,h] + h_tgt[b,t,r,h]) * w_out[h]
  mask[b, tt, l]    = sum_h tanh((hidden @ dense_W.T + dense_b)[b,l,tt,h]) * clf_W[0,h] + clf_b

Sharding: 8 cores <-> (b, r) in {0,1} x {0..3}.  Each core computes the full
L x L pairwise block for its (b, r) entirely on-chip (the (B,L,L,R,H)
intermediate never touches HBM).  The tiny dense head is split by h-range
(192 per core within each batch); host sums the partials.

Per-core engine plan:
  PE : projections (h_srcT/h_tgtT, layout [h, token]) + w_out contraction
       (M=1 accumulating matmuls over 6 h-chunks)
  DVE: pairwise broadcast-add  pair[h, s, t] = srcT[h,s] + tgtT[h,t]
  ACT: tanh over the pair tiles (the throughput floor: ~12.6M elem/core)
"""

import sys

if "/opt/trn_rl_repo" not in sys.path:
    sys.path.insert(0, "/opt/trn_rl_repo")

import numpy as np

B, L, H, R, T = 2, 128, 768, 4, 3
P = 128
KC = H // P            # 6 h-chunks of 128
SB = 32                # source-positions per pairwise block
NSB = L // SB          # 4 blocks
NSUB = 8               # 512-wide reduce sub-blocks per pairwise block
HRANGE = H // 4        # dense-head h columns per core (4 cores per batch)
DM = 96                # dense-head M-chunk (HRANGE*T rows = 576 = 6 x 96)
NDM = (HRANGE * T) // DM

_cache = {}


def _build():
    import concourse.bacc as bacc
    import concourse.tile as tile
    from concourse import mybir

    f32 = mybir.dt.float32
    f32r = mybir.dt.float32r
    AF = mybir.ActivationFunctionType
    OP = mybir.AluOpType

    nc = bacc.Bacc("TRN2", target_bir_lowering=False, debug=False)

    # ---- DRAM I/O (per-core views; host pre-transposes/pre-tiles) ----
    hidT_d = nc.dram_tensor("hidT", (KC, P, L), f32, kind="ExternalInput")
    wsrcT_d = nc.dram_tensor("wsrcT", (KC, KC, P, P), f32, kind="ExternalInput")
    wtgtT_d = nc.dram_tensor("wtgtT", (KC, KC, P, P), f32, kind="ExternalInput")
    bsrc_d = nc.dram_tensor("bsrc", (1, H), f32, kind="ExternalInput")
    btgt_d = nc.dram_tensor("btgt", (1, H), f32, kind="ExternalInput")
    wout_d = nc.dram_tensor("wout", (P, KC), f32, kind="ExternalInput")
    dwT_d = nc.dram_tensor("dwT", (KC, NDM, P, DM), f32, kind="ExternalInput")
    db_d = nc.dram_tensor("db", (DM, NDM), f32, kind="ExternalInput")
    clf_d = nc.dram_tensor("clf", (DM, NDM // T), f32, kind="ExternalInput")

    rel_d = nc.dram_tensor("rel", (NSB * NSUB, 512), f32, kind="ExternalOutput")
    hsrc_d = nc.dram_tensor("hsrc", (KC, P, L), f32, kind="ExternalOutput")
    htgt_d = nc.dram_tensor("htgt", (KC, P, L), f32, kind="ExternalOutput")
    maskp_d = nc.dram_tensor("maskp", (T, L), f32, kind="ExternalOutput")

    with tile.TileContext(nc) as tc:
        with (
            tc.tile_pool(name="consts", bufs=1) as consts,
            tc.tile_pool(name="wpool", bufs=8) as wpool,
            tc.tile_pool(name="projsb", bufs=1) as projsb,
            tc.tile_pool(name="pairp", bufs=7) as pairp,
            tc.tile_pool(name="evacp", bufs=4) as evacp,
            tc.tile_pool(name="mmps", bufs=2, space="PSUM") as mmps,
            tc.tile_pool(name="rowps", bufs=3, space="PSUM") as rowps,
        ):
            # ---------------- constants ----------------
            ones = consts.tile([1, P], f32, name="ones", tag="ones")
            nc.vector.memset(ones[:], 1.0)

            hidT = consts.tile([P, KC * P], f32, name="hidT_sb", tag="hidT_sb")
            for kc in range(KC):
                nc.sync.dma_start(hidT[:, kc * P:(kc + 1) * P], hidT_d[kc])

            bsrc = consts.tile([1, H], f32, name="bsrc_sb", tag="bsrc_sb")
            nc.sync.dma_start(bsrc[:], bsrc_d[:])
            btgt = consts.tile([1, H], f32, name="btgt_sb", tag="btgt_sb")
            nc.sync.dma_start(btgt[:], btgt_d[:])
            wout = consts.tile([P, KC], f32, name="wout_sb", tag="wout_sb")
            nc.sync.dma_start(wout[:], wout_d[:])
            # f32r-rounded copy of w_out for the 1-pass reduce matmuls
            wout_r = consts.tile([P, KC], f32r, name="wout_r", tag="wout_r")
            nc.vector.tensor_copy(wout_r[:], wout[:])
            db = consts.tile([DM, NDM], f32, name="db_sb", tag="db_sb")
            nc.sync.dma_start(db[:], db_d[:])
            clf = consts.tile([DM, NDM // T], f32, name="clf_sb", tag="clf_sb")
            nc.sync.dma_start(clf[:], clf_d[:])

            srcT = projsb.tile([P, KC * P], f32, name="srcT", tag="srcT")
            tgtT = projsb.tile([P, KC * P], f32, name="tgtT", tag="tgtT")

            # ---------------- projections (PE) ----------------
            # h_xT[m-chunk] = sum_kc W_xT[kc,m].T @ hidT[kc]  + b_x (rank-1)
            for m in range(KC):
                for w_d, b_sb, outT, out_d, nm in (
                    (wsrcT_d, bsrc, srcT, hsrc_d, "s"),
                    (wtgtT_d, btgt, tgtT, htgt_d, "t"),
                ):
                    ps = mmps.tile([P, P], f32, name=f"ps_{nm}{m}", tag="proj")
                    for kc in range(KC):
                        wt = wpool.tile([P, P], f32, name=f"w_{nm}{m}_{kc}", tag="wt")
                        nc.sync.dma_start(wt[:], w_d[kc, m])
                        nc.tensor.matmul(
                            ps[:], wt[:], hidT[:, kc * P:(kc + 1) * P],
                            start=(kc == 0), stop=False,
                        )
                    nc.tensor.matmul(
                        ps[:], b_sb[:, m * P:(m + 1) * P], ones[:],
                        start=False, stop=True,
                    )
                    nc.scalar.copy(outT[:, m * P:(m + 1) * P], ps[:])
                    nc.sync.dma_start(out_d[m], outT[:, m * P:(m + 1) * P])

            # ---------------- dense head (tiny) ----------------
            zt = projsb.tile([DM, NDM * P], f32, name="zt", tag="zt")
            for m in range(NDM):
                psd = mmps.tile([DM, P], f32, name=f"psd{m}", tag="dense")
                for kc in range(KC):
                    wt = wpool.tile([P, DM], f32, name=f"wd{m}_{kc}", tag="wtd")
                    nc.sync.dma_start(wt[:], dwT_d[kc, m])
                    nc.tensor.matmul(
                        psd[:], wt[:], hidT[:, kc * P:(kc + 1) * P],
                        start=(kc == 0), stop=(kc == KC - 1),
                    )
                nc.scalar.activation(
                    zt[:, m * P:(m + 1) * P], psd[:], AF.Tanh,
                    bias=db[:, m:m + 1],
                )
            for tt in range(T):
                pm = rowps.tile([1, P], f32, name=f"pm{tt}", tag="row")
                for j in range(NDM // T):
                    m = tt * (NDM // T) + j
                    nc.tensor.matmul(
                        pm[:], clf[:, j:j + 1], zt[:, m * P:(m + 1) * P],
                        start=(j == 0), stop=(j == NDM // T - 1),
                    )
                ev = evacp.tile([1, P], f32, name=f"mev{tt}", tag="ev")
                nc.vector.tensor_copy(ev[:], pm[:])
                nc.sync.dma_start(maskp_d[tt], ev[:])

            # ---------------- pairwise (DVE add -> ACT tanh -> PE reduce) ----
            for sb in range(NSB):
                s0 = sb * SB
                ptiles = []
                for kc in range(KC):
                    pt = pairp.tile([P, SB, P], f32r, name=f"pair{sb}_{kc}", tag="pair")
                    nc.vector.tensor_tensor(
                        pt[:],
                        srcT[:, kc * P + s0: kc * P + s0 + SB][:, :, None]
                        .to_broadcast((P, SB, P)),
                        tgtT[:, None, kc * P:(kc + 1) * P]
                        .to_broadcast((P, SB, P)),
                        op=OP.add,
                    )
                    nc.scalar.activation(pt[:], pt[:], AF.Tanh)
                    ptiles.append(pt)
                for sub in range(NSUB):
                    sl = sub * 4          # 4 source positions per 512-col slab
                    pr = rowps.tile([1, 512], f32, name=f"pr{sb}_{sub}", tag="row")
                    for kc in range(KC):
                        # float32r: one PE pass (1 cyc/row at N>=256) instead
                        # of fp32's two half-speed passes.
                        nc.tensor.matmul(
                            pr[:], wout_r[:, kc:kc + 1],
                            ptiles[kc][:, sl:sl + 4, :],
                            start=(kc == 0), stop=(kc == KC - 1),
                        )
                    ev = evacp.tile([1, 512], f32, name=f"rev{sb}_{sub}", tag="ev")
                    if (sb * NSUB + sub) % 2 == 0:
                        nc.vector.tensor_copy(ev[:], pr[:])
                    else:
                        nc.scalar.copy(ev[:], pr[:])
                    nc.sync.dma_start(rel_d[sb * NSUB + sub], ev[:])

    nc.compile()
    return nc


def _in_maps(inputs):
    hidden = np.asarray(inputs["hidden_state"], np.float32)
    W_src = np.asarray(inputs["W_src"], np.float32)
    b_src = np.asarray(inputs["b_src"], np.float32)
    W_tgt = np.asarray(inputs["W_tgt"], np.float32)
    b_tgt = np.asarray(inputs["b_tgt"], np.float32)
    w_out = np.asarray(inputs["w_out"], np.float32)
    dense_W = np.asarray(inputs["dense_W"], np.float32)
    dense_b = np.asarray(inputs["dense_b"], np.float32)
    clf_W = np.asarray(inputs["clf_W"], np.float32)

    def tile_wT(w_block, mdim):
        # [rows, H] weight block -> lhsT chunks (kc, m, k, mcol)
        wT = np.ascontiguousarray(w_block.T)          # [H(k), rows(m)]
        nm = w_block.shape[0] // mdim
        return np.ascontiguousarray(
            wT.reshape(KC, P, nm, mdim).transpose(0, 2, 1, 3)
        )

    wout_t = np.ascontiguousarray(w_out.reshape(KC, P).T)  # [128, 6]

    maps = []
    for c in range(8):
        b, r = c // 4, c % 4
        hr0 = (c % 4) * HRANGE
        rows = np.concatenate(
            [np.arange(tt * H + hr0, tt * H + hr0 + HRANGE) for tt in range(T)]
        )
        maps.append({
            "hidT": np.ascontiguousarray(hidden[b].T).reshape(KC, P, L),
            "wsrcT": tile_wT(W_src[r * H:(r + 1) * H], P),
            "wtgtT": tile_wT(W_tgt[r * H:(r + 1) * H], P),
            "bsrc": np.ascontiguousarray(b_src[r * H:(r + 1) * H]).reshape(1, H),
            "btgt": np.ascontiguousarray(b_tgt[r * H:(r + 1) * H]).reshape(1, H),
            "wout": wout_t,
            "dwT": tile_wT(dense_W[rows], DM),
            "db": np.ascontiguousarray(dense_b[rows].reshape(NDM, DM).T),
            "clf": np.ascontiguousarray(clf_W[0, hr0:hr0 + HRANGE].reshape(NDM // T, DM).T),
        })
    return maps


def _assemble(results, inputs):
    clf_b = np.asarray(inputs["clf_b"], np.float32)
    rel = np.empty((B, R, L, L), np.float32)
    h_src = np.empty((B, L, R, H), np.float32)
    h_tgt = np.empty((B, L, R, H), np.float32)
    mask = np.zeros((B, T, L), np.float32)
    for c in range(8):
        b, r = c // 4, c % 4
        out = results[c]
        rel[b, r] = out["rel"].reshape(L, L)
        h_src[b, :, r, :] = out["hsrc"].transpose(2, 0, 1).reshape(L, H)
        h_tgt[b, :, r, :] = out["htgt"].transpose(2, 0, 1).reshape(L, H)
        mask[b] += out["maskp"]
    mask += clf_b[0]
    return rel, mask, h_src, h_tgt


def _run(inputs, trace=False):
    from concourse import bass_utils

    if "nc" not in _cache:
        _cache["nc"] = _build()
    res = bass_utils.run_bass_kernel_spmd(
        _cache["nc"], _in_maps(inputs), core_ids=list(range(8)), trace=trace,
    )
    return _assemble(res.results, inputs), res


def kernel(**inputs):
    out, _ = _run(inputs, trace=False)
    return out


# revision 13
# speedup vs baseline: 1.4166x; 1.0705x over previous
"""Trainium2 Bass kernel for the biaffine pairwise relation scorer.

Model (per reference):
  h_src = (hidden @ W_src.T + b_src).reshape(B, L, R, H)
  h_tgt = (hidden @ W_tgt.T + b_tgt).reshape(B, L, R, H)
  rel[b, r, s, t]   = sum_h tanh(h_src[b,s,r,h] + h_tgt[b,t,r,h]) * w_out[h]
  mask[b, tt, l]    = sum_h tanh((hidden @ dense_W.T + dense_b)[b,l,tt,h]) * clf_W[0,h] + clf_b

Sharding: 8 cores <-> (b, r) in {0,1} x {0..3}.  Each core computes the full
L x L pairwise block for its (b, r) entirely on-chip (the (B,L,L,R,H)
intermediate never touches HBM).  The tiny dense head is split by h-range
(192 per core within each batch); host sums the partials.

Per-core engine plan:
  PE : projections (h_srcT/h_tgtT, layout [h, token]) + w_out contraction
       (M=1 accumulating matmuls over 6 h-chunks)
  DVE: pairwise broadcast-add  pair[h, s, t] = srcT[h,s] + tgtT[h,t]
  ACT: tanh over the pair tiles (the throughput floor: ~12.6M elem/core)
"""

import sys

if "/opt/trn_rl_repo" not in sys.path:
    sys.path.insert(0, "/opt/trn_rl_repo")

import numpy as np

B, L, H, R, T = 2, 128, 768, 4, 3
P = 128
KC = H // P            # 6 h-chunks of 128
SB = 16                # source-positions per pairwise block
NSB = L // SB          # 8 blocks
NSUB = 4               # 512-wide reduce sub-blocks per pairwise block
HRANGE = H // 4        # dense-head h columns per core (4 cores per batch)
DM = 96                # dense-head M-chunk (HRANGE*T rows = 576 = 6 x 96)
NDM = (HRANGE * T) // DM

_cache = {}


def _build():
    import concourse.bacc as bacc
    import concourse.tile as tile
    from concourse import mybir

    f32 = mybir.dt.float32
    f32r = mybir.dt.float32r
    AF = mybir.ActivationFunctionType
    OP = mybir.AluOpType

    nc = bacc.Bacc("TRN2", target_bir_lowering=False, debug=False)

    # ---- DRAM I/O (per-core views; host pre-transposes/pre-tiles) ----
    hidT_d = nc.dram_tensor("hidT", (KC, P, L), f32, kind="ExternalInput")
    wsrcT_d = nc.dram_tensor("wsrcT", (KC, KC, P, P), f32, kind="ExternalInput")
    wtgtT_d = nc.dram_tensor("wtgtT", (KC, KC, P, P), f32, kind="ExternalInput")
    bsrc_d = nc.dram_tensor("bsrc", (1, H), f32, kind="ExternalInput")
    btgt_d = nc.dram_tensor("btgt", (1, H), f32, kind="ExternalInput")
    wout_d = nc.dram_tensor("wout", (P, KC), f32, kind="ExternalInput")
    dwT_d = nc.dram_tensor("dwT", (KC, NDM, P, DM), f32, kind="ExternalInput")
    db_d = nc.dram_tensor("db", (DM, NDM), f32, kind="ExternalInput")
    clf_d = nc.dram_tensor("clf", (DM, NDM // T), f32, kind="ExternalInput")

    rel_d = nc.dram_tensor("rel", (NSB * NSUB, 512), f32, kind="ExternalOutput")
    hsrc_d = nc.dram_tensor("hsrc", (KC, P, L), f32, kind="ExternalOutput")
    htgt_d = nc.dram_tensor("htgt", (KC, P, L), f32, kind="ExternalOutput")
    maskp_d = nc.dram_tensor("maskp", (T, L), f32, kind="ExternalOutput")

    with tile.TileContext(nc) as tc:
        with (
            tc.tile_pool(name="consts", bufs=1) as consts,
            tc.tile_pool(name="wpool", bufs=8) as wpool,
            tc.tile_pool(name="projsb", bufs=1) as projsb,
            tc.tile_pool(name="pairp", bufs=13) as pairp,
            tc.tile_pool(name="evacp", bufs=6) as evacp,
            tc.tile_pool(name="mmps", bufs=2, space="PSUM") as mmps,
            tc.tile_pool(name="rowps", bufs=6, space="PSUM") as rowps,
        ):
            # ---------------- constants ----------------
            ones = consts.tile([1, P], f32, name="ones", tag="ones")
            nc.vector.memset(ones[:], 1.0)

            hidT = consts.tile([P, KC * P], f32, name="hidT_sb", tag="hidT_sb")
            for kc in range(KC):
                nc.sync.dma_start(hidT[:, kc * P:(kc + 1) * P], hidT_d[kc])

            bsrc = consts.tile([1, H], f32, name="bsrc_sb", tag="bsrc_sb")
            nc.sync.dma_start(bsrc[:], bsrc_d[:])
            btgt = consts.tile([1, H], f32, name="btgt_sb", tag="btgt_sb")
            nc.sync.dma_start(btgt[:], btgt_d[:])
            wout = consts.tile([P, KC], f32, name="wout_sb", tag="wout_sb")
            nc.sync.dma_start(wout[:], wout_d[:])
            # f32r-rounded copy of w_out for the 1-pass reduce matmuls
            wout_r = consts.tile([P, KC], f32r, name="wout_r", tag="wout_r")
            nc.vector.tensor_copy(wout_r[:], wout[:])
            db = consts.tile([DM, NDM], f32, name="db_sb", tag="db_sb")
            nc.sync.dma_start(db[:], db_d[:])
            clf = consts.tile([DM, NDM // T], f32, name="clf_sb", tag="clf_sb")
            nc.sync.dma_start(clf[:], clf_d[:])

            srcT = projsb.tile([P, KC * P], f32, name="srcT", tag="srcT")
            tgtT = projsb.tile([P, KC * P], f32, name="tgtT", tag="tgtT")

            # ---------------- projections (PE) ----------------
            # h_xT[m-chunk] = sum_kc W_xT[kc,m].T @ hidT[kc]  + b_x (rank-1)
            for m in range(KC):
                for w_d, b_sb, outT, out_d, nm in (
                    (wsrcT_d, bsrc, srcT, hsrc_d, "s"),
                    (wtgtT_d, btgt, tgtT, htgt_d, "t"),
                ):
                    ps = mmps.tile([P, P], f32, name=f"ps_{nm}{m}", tag="mm")
                    for kc in range(KC):
                        wt = wpool.tile([P, P], f32, name=f"w_{nm}{m}_{kc}", tag="wt")
                        nc.gpsimd.dma_start(wt[:], w_d[kc, m])
                        nc.tensor.matmul(
                            ps[:], wt[:], hidT[:, kc * P:(kc + 1) * P],
                            start=(kc == 0), stop=False,
                        )
                    nc.tensor.matmul(
                        ps[:], b_sb[:, m * P:(m + 1) * P], ones[:],
                        start=False, stop=True,
                    )
                    nc.scalar.copy(outT[:, m * P:(m + 1) * P], ps[:])
                    nc.sync.dma_start(out_d[m], outT[:, m * P:(m + 1) * P])

            # ---------------- dense head (tiny) ----------------
            zt = projsb.tile([DM, NDM * P], f32, name="zt", tag="zt")
            for m in range(NDM):
                psd = mmps.tile([DM, P], f32, name=f"psd{m}", tag="mm")
                for kc in range(KC):
                    wt = wpool.tile([P, DM], f32, name=f"wd{m}_{kc}", tag="wtd")
                    nc.gpsimd.dma_start(wt[:], dwT_d[kc, m])
                    nc.tensor.matmul(
                        psd[:], wt[:], hidT[:, kc * P:(kc + 1) * P],
                        start=(kc == 0), stop=(kc == KC - 1),
                    )
                nc.scalar.activation(
                    zt[:, m * P:(m + 1) * P], psd[:], AF.Tanh,
                    bias=db[:, m:m + 1],
                )
            for tt in range(T):
                pm = rowps.tile([1, P], f32, name=f"pm{tt}", tag="row")
                for j in range(NDM // T):
                    m = tt * (NDM // T) + j
                    nc.tensor.matmul(
                        pm[:], clf[:, j:j + 1], zt[:, m * P:(m + 1) * P],
                        start=(j == 0), stop=(j == NDM // T - 1),
                    )
                ev = evacp.tile([1, P], f32, name=f"mev{tt}", tag="ev")
                nc.vector.tensor_copy(ev[:], pm[:])
                nc.sync.dma_start(maskp_d[tt], ev[:])

            # ---------------- pairwise (DVE add -> ACT tanh -> PE reduce) ----
            for sb in range(NSB):
                s0 = sb * SB
                ptiles = []
                for kc in range(KC):
                    pt = pairp.tile([P, SB, P], f32r, name=f"pair{sb}_{kc}", tag="pair")
                    # spread the broadcast-adds: 1 in 4 goes to the otherwise
                    # idle GpSimd engine
                    add_eng = nc.gpsimd if (sb * KC + kc) % 4 == 3 else nc.vector
                    add_eng.tensor_tensor(
                        pt[:],
                        srcT[:, kc * P + s0: kc * P + s0 + SB][:, :, None]
                        .to_broadcast((P, SB, P)),
                        tgtT[:, None, kc * P:(kc + 1) * P]
                        .to_broadcast((P, SB, P)),
                        op=OP.add,
                    )
                    nc.scalar.activation(pt[:], pt[:], AF.Tanh)
                    ptiles.append(pt)
                for sub in range(NSUB):
                    sl = sub * 4          # 4 source positions per 512-col slab
                    pr = rowps.tile([1, 512], f32, name=f"pr{sb}_{sub}", tag="row")
                    for kc in range(KC):
                        # float32r: one PE pass (1 cyc/row at N>=256) instead
                        # of fp32's two half-speed passes.
                        nc.tensor.matmul(
                            pr[:], wout_r[:, kc:kc + 1],
                            ptiles[kc][:, sl:sl + 4, :],
                            start=(kc == 0), stop=(kc == KC - 1),
                        )
                    ev = evacp.tile([1, 512], f32, name=f"rev{sb}_{sub}", tag="ev")
                    if (sb * NSUB + sub) % 2 == 0:
                        nc.vector.tensor_copy(ev[:], pr[:])
                    else:
                        nc.scalar.copy(ev[:], pr[:])
                    nc.sync.dma_start(rel_d[sb * NSUB + sub], ev[:])

    nc.compile()
    return nc


def _in_maps(inputs):
    hidden = np.asarray(inputs["hidden_state"], np.float32)
    W_src = np.asarray(inputs["W_src"], np.float32)
    b_src = np.asarray(inputs["b_src"], np.float32)
    W_tgt = np.asarray(inputs["W_tgt"], np.float32)
    b_tgt = np.asarray(inputs["b_tgt"], np.float32)
    w_out = np.asarray(inputs["w_out"], np.float32)
    dense_W = np.asarray(inputs["dense_W"], np.float32)
    dense_b = np.asarray(inputs["dense_b"], np.float32)
    clf_W = np.asarray(inputs["clf_W"], np.float32)

    def tile_wT(w_block, mdim):
        # [rows, H] weight block -> lhsT chunks (kc, m, k, mcol)
        wT = np.ascontiguousarray(w_block.T)          # [H(k), rows(m)]
        nm = w_block.shape[0] // mdim
        return np.ascontiguousarray(
            wT.reshape(KC, P, nm, mdim).transpose(0, 2, 1, 3)
        )

    wout_t = np.ascontiguousarray(w_out.reshape(KC, P).T)  # [128, 6]

    maps = []
    for c in range(8):
        b, r = c // 4, c % 4
        hr0 = (c % 4) * HRANGE
        rows = np.concatenate(
            [np.arange(tt * H + hr0, tt * H + hr0 + HRANGE) for tt in range(T)]
        )
        maps.append({
            "hidT": np.ascontiguousarray(hidden[b].T).reshape(KC, P, L),
            "wsrcT": tile_wT(W_src[r * H:(r + 1) * H], P),
            "wtgtT": tile_wT(W_tgt[r * H:(r + 1) * H], P),
            "bsrc": np.ascontiguousarray(b_src[r * H:(r + 1) * H]).reshape(1, H),
            "btgt": np.ascontiguousarray(b_tgt[r * H:(r + 1) * H]).reshape(1, H),
            "wout": wout_t,
            "dwT": tile_wT(dense_W[rows], DM),
            "db": np.ascontiguousarray(dense_b[rows].reshape(NDM, DM).T),
            "clf": np.ascontiguousarray(clf_W[0, hr0:hr0 + HRANGE].reshape(NDM // T, DM).T),
        })
    return maps


def _assemble(results, inputs):
    clf_b = np.asarray(inputs["clf_b"], np.float32)
    rel = np.empty((B, R, L, L), np.float32)
    h_src = np.empty((B, L, R, H), np.float32)
    h_tgt = np.empty((B, L, R, H), np.float32)
    mask = np.zeros((B, T, L), np.float32)
    for c in range(8):
        b, r = c // 4, c % 4
        out = results[c]
        rel[b, r] = out["rel"].reshape(L, L)
        h_src[b, :, r, :] = out["hsrc"].transpose(2, 0, 1).reshape(L, H)
        h_tgt[b, :, r, :] = out["htgt"].transpose(2, 0, 1).reshape(L, H)
        mask[b] += out["maskp"]
    mask += clf_b[0]
    return rel, mask, h_src, h_tgt


def _run(inputs, trace=False):
    from concourse import bass_utils

    if "nc" not in _cache:
        _cache["nc"] = _build()
    res = bass_utils.run_bass_kernel_spmd(
        _cache["nc"], _in_maps(inputs), core_ids=list(range(8)), trace=trace,
    )
    return _assemble(res.results, inputs), res


def kernel(**inputs):
    out, _ = _run(inputs, trace=False)
    return out


# revision 17
# speedup vs baseline: 1.6531x; 1.1670x over previous
"""Trainium2 Bass kernel for the biaffine pairwise relation scorer.

Model (per reference):
  h_src = (hidden @ W_src.T + b_src).reshape(B, L, R, H)
  h_tgt = (hidden @ W_tgt.T + b_tgt).reshape(B, L, R, H)
  rel[b, r, s, t]   = sum_h tanh(h_src[b,s,r,h] + h_tgt[b,t,r,h]) * w_out[h]
  mask[b, tt, l]    = sum_h tanh((hidden @ dense_W.T + dense_b)[b,l,tt,h]) * clf_W[0,h] + clf_b

Sharding: 8 cores <-> (b, r) in {0,1} x {0..3}.  Each core computes the full
L x L pairwise block for its (b, r) entirely on-chip (the (B,L,L,R,H)
intermediate never touches HBM).  The tiny dense head is split by h-range
(192 per core within each batch); host sums the partials.

Per-core engine plan:
  PE : projections as N=512 matmuls with hidT stationary (natural [token, rh]
       output), PE transposes for the [h, token] layouts, and the w_out
       contraction as f32r (1 cyc/row) accumulating matmuls.  The four 512-col
       reduce sub-blocks of a source-block land on psum partitions 0/32/64/96
       of ONE bank via zero-padded prefix lhsT tiles (descending-M order), so
       evacuation is a single dense 128-partition copy.
  DVE: pairwise broadcast-add  pair[h, s, t] = srcT[h,s] + tgtT[h,t]
  ACT: tanh over the pair tiles (the throughput floor: ~12.6M elem/core)
"""

import sys

if "/opt/trn_rl_repo" not in sys.path:
    sys.path.insert(0, "/opt/trn_rl_repo")

import numpy as np

B, L, H, R, T = 2, 128, 768, 4, 3
P = 128
KC = H // P            # 6 h-chunks of 128
SB = 16                # source-positions per pairwise block
NSB = L // SB          # 8 blocks
NSUB = 4               # 512-wide reduce sub-blocks per pairwise block
WZ = 97                # prefix-Z lhsT width (w_out at cols 0/32/64/96)
HRANGE = H // 4        # dense-head h columns per core (4 cores per batch)
DROWS = HRANGE * T     # 576 dense rows per core
NZC = (DROWS + P - 1) // P  # 5 z transpose/reduce chunks (4x128 + 64)

_cache = {}


def _build():
    import concourse.bacc as bacc
    import concourse.tile as tile
    from concourse import mybir
    from concourse.masks import make_identity

    f32 = mybir.dt.float32
    f32r = mybir.dt.float32r
    AF = mybir.ActivationFunctionType
    OP = mybir.AluOpType

    nc = bacc.Bacc("TRN2", target_bir_lowering=False, debug=False)

    # ---- DRAM I/O (per-core views; host pre-transposes/pre-tiles) ----
    hidT_d = nc.dram_tensor("hidT", (KC, P, L), f32, kind="ExternalInput")
    wsrcT_d = nc.dram_tensor("wsrcT", (KC, P, H), f32, kind="ExternalInput")
    wtgtT_d = nc.dram_tensor("wtgtT", (KC, P, H), f32, kind="ExternalInput")
    bsrc_d = nc.dram_tensor("bsrc", (1, H), f32, kind="ExternalInput")
    btgt_d = nc.dram_tensor("btgt", (1, H), f32, kind="ExternalInput")
    woutZ_d = nc.dram_tensor("woutZ", (P, KC * WZ), f32, kind="ExternalInput")
    dwT_d = nc.dram_tensor("dwT", (KC, P, DROWS), f32, kind="ExternalInput")
    db_d = nc.dram_tensor("db", (1, DROWS), f32, kind="ExternalInput")
    clfZ_d = nc.dram_tensor("clfZ", (NZC, P, T), f32, kind="ExternalInput")

    rel_d = nc.dram_tensor("rel", (NSB * NSUB, 512), f32, kind="ExternalOutput")
    hsrc_d = nc.dram_tensor("hsrc", (P, H), f32, kind="ExternalOutput")
    htgt_d = nc.dram_tensor("htgt", (P, H), f32, kind="ExternalOutput")
    maskp_d = nc.dram_tensor("maskp", (T, L), f32, kind="ExternalOutput")

    with tile.TileContext(nc) as tc:
        with (
            tc.tile_pool(name="consts", bufs=1) as consts,
            tc.tile_pool(name="wpool", bufs=4) as wpool,
            tc.tile_pool(name="projsb", bufs=1) as projsb,
            tc.tile_pool(name="pairp", bufs=13) as pairp,
            tc.tile_pool(name="evacp", bufs=6) as evacp,
            tc.tile_pool(name="mmps", bufs=2, space="PSUM") as mmps,
            tc.tile_pool(name="rowps", bufs=3, space="PSUM") as rowps,
        ):
            # ---------------- constants ----------------
            ones = consts.tile([1, P], f32, name="ones", tag="ones")
            nc.vector.memset(ones[:], 1.0)
            ident = consts.tile([P, P], f32, name="ident", tag="ident")
            make_identity(nc, ident[:])

            hidT = consts.tile([P, KC * P], f32, name="hidT_sb", tag="hidT_sb")
            for kc in range(KC):
                nc.sync.dma_start(hidT[:, kc * P:(kc + 1) * P], hidT_d[kc])

            bsrc = consts.tile([1, H], f32, name="bsrc_sb", tag="bsrc_sb")
            nc.sync.dma_start(bsrc[:], bsrc_d[:])
            btgt = consts.tile([1, H], f32, name="btgt_sb", tag="btgt_sb")
            nc.sync.dma_start(btgt[:], btgt_d[:])
            db = consts.tile([1, DROWS], f32, name="db_sb", tag="db_sb")
            nc.sync.dma_start(db[:], db_d[:])
            woutZ = consts.tile([P, KC * WZ], f32, name="woutZ_sb", tag="woutZ_sb")
            nc.sync.dma_start(woutZ[:], woutZ_d[:])
            woutZr = consts.tile([P, KC * WZ], f32r, name="woutZr", tag="woutZr")
            nc.vector.tensor_copy(woutZr[:], woutZ[:])
            clfZ = consts.tile([P, NZC * T], f32, name="clfZ_sb", tag="clfZ_sb")
            for c in range(NZC):
                nc.sync.dma_start(clfZ[:, c * T:(c + 1) * T], clfZ_d[c])

            srcT = projsb.tile([P, KC * P], f32, name="srcT", tag="srcT")
            tgtT = projsb.tile([P, KC * P], f32, name="tgtT", tag="tgtT")
            srcn = projsb.tile([P, H], f32, name="srcn", tag="srcn")
            tgtn = projsb.tile([P, H], f32, name="tgtn", tag="tgtn")
            ztn = projsb.tile([P, DROWS], f32, name="ztn", tag="ztn")
            ztT = projsb.tile([P, NZC * P], f32, name="ztT", tag="ztT")

            # ---------------- projections (PE, natural layout) ----------
            # h_x[token, rh] = sum_kc hidT[kc].T @ W_xT[kc]  (+ rank-1 bias)
            src_ps = mmps.tile([P, H], f32, name="src_ps", tag="bigps")
            tgt_ps = mmps.tile([P, H], f32, name="tgt_ps", tag="bigps")
            for kc in range(KC):
                wts = wpool.tile([P, H], f32, name=f"ws{kc}", tag="w")
                nc.gpsimd.dma_start(wts[:], wsrcT_d[kc])
                wtt = wpool.tile([P, H], f32, name=f"wt{kc}", tag="w")
                nc.gpsimd.dma_start(wtt[:], wtgtT_d[kc])
                hk = hidT[:, kc * P:(kc + 1) * P]
                for n0 in (0, 512):
                    n1 = min(n0 + 512, H)
                    nc.tensor.matmul(src_ps[:, n0:n1], hk, wts[:, n0:n1],
                                     start=(kc == 0), stop=False)
                    nc.tensor.matmul(tgt_ps[:, n0:n1], hk, wtt[:, n0:n1],
                                     start=(kc == 0), stop=False)
            for n0 in (0, 512):
                n1 = min(n0 + 512, H)
                nc.tensor.matmul(src_ps[:, n0:n1], ones[:], bsrc[:, n0:n1],
                                 start=False, stop=True)
                nc.tensor.matmul(tgt_ps[:, n0:n1], ones[:], btgt[:, n0:n1],
                                 start=False, stop=True)
            nc.scalar.copy(srcn[:], src_ps[:])
            nc.sync.dma_start(hsrc_d[:], srcn[:])
            nc.scalar.copy(tgtn[:], tgt_ps[:])
            nc.sync.dma_start(htgt_d[:], tgtn[:])

            # transpose to [h, token] for the pairwise stage
            for src_of, dst in ((srcn, srcT), (tgtn, tgtT)):
                for kc in range(KC):
                    tp = rowps.tile([P, P], f32, name=f"tp_{dst.name}{kc}", tag="row")
                    nc.tensor.transpose(tp[:], src_of[:, kc * P:(kc + 1) * P], ident[:])
                    nc.scalar.copy(dst[:, kc * P:(kc + 1) * P], tp[:])

            # ---------------- dense head (tiny) ----------------
            z_ps = mmps.tile([P, DROWS], f32, name="z_ps", tag="bigps")
            for kc in range(KC):
                wtd = wpool.tile([P, DROWS], f32, name=f"wd{kc}", tag="w")
                nc.gpsimd.dma_start(wtd[:], dwT_d[kc])
                hk = hidT[:, kc * P:(kc + 1) * P]
                for n0 in (0, 512):
                    n1 = min(n0 + 512, DROWS)
                    nc.tensor.matmul(z_ps[:, n0:n1], hk, wtd[:, n0:n1],
                                     start=(kc == 0), stop=False)
            for n0 in (0, 512):
                n1 = min(n0 + 512, DROWS)
                nc.tensor.matmul(z_ps[:, n0:n1], ones[:], db[:, n0:n1],
                                 start=False, stop=True)
            nc.scalar.activation(ztn[:], z_ps[:], AF.Tanh)
            for c in range(NZC):
                w = min(P, DROWS - c * P)
                tp = rowps.tile([P, P], f32, name=f"tpz{c}", tag="row")
                nc.tensor.transpose(tp[:w, :], ztn[:, c * P:c * P + w], ident[:])
                nc.scalar.copy(ztT[:w, c * P:(c + 1) * P], tp[:w, :])
            pm = rowps.tile([T, L], f32, name="pm", tag="row")
            for c in range(NZC):
                w = min(P, DROWS - c * P)
                nc.tensor.matmul(pm[:], clfZ[:w, c * T:(c + 1) * T],
                                 ztT[:w, c * P:(c + 1) * P],
                                 start=(c == 0), stop=(c == NZC - 1))
            mev = evacp.tile([T, L], f32, name="mev", tag="ev")
            nc.vector.tensor_copy(mev[:], pm[:])
            nc.sync.dma_start(maskp_d[:], mev[:])

            # ---------------- pairwise (DVE add -> ACT tanh -> PE reduce) ----
            for sb in range(NSB):
                s0 = sb * SB
                ptiles = []
                for kc in range(KC):
                    pt = pairp.tile([P, SB, P], f32r, name=f"pair{sb}_{kc}", tag="pair")
                    nc.vector.tensor_tensor(
                        pt[:],
                        srcT[:, kc * P + s0: kc * P + s0 + SB][:, :, None]
                        .to_broadcast((P, SB, P)),
                        tgtT[:, None, kc * P:(kc + 1) * P]
                        .to_broadcast((P, SB, P)),
                        op=OP.add,
                    )
                    nc.scalar.activation(pt[:], pt[:], AF.Tanh)
                    ptiles.append(pt)
                # All four 512-col sub-blocks accumulate in ONE psum bank, on
                # partitions 96/64/32/0 (descending prefix width: later,
                # narrower groups reset the rows the earlier ones smeared).
                pp = rowps.tile([P, 512], f32, name=f"pp{sb}", tag="row")
                for j in (3, 2, 1, 0):
                    m = 32 * j + 1
                    for kc in range(KC):
                        nc.tensor.matmul(
                            pp[:m, :], woutZr[:, kc * WZ: kc * WZ + m],
                            ptiles[kc][:, j * 4:(j + 1) * 4, :],
                            start=(kc == 0), stop=(kc == KC - 1),
                        )
                ev = evacp.tile([P, 512], f32, name=f"rev{sb}", tag="ev")
                nc.vector.tensor_copy(ev[:WZ, :], pp[:WZ, :])
                nc.sync.dma_start(rel_d[sb * NSUB:(sb + 1) * NSUB, :],
                                  ev[0:WZ:32, :])

    nc.compile()
    return nc


def _in_maps(inputs):
    hidden = np.asarray(inputs["hidden_state"], np.float32)
    W_src = np.asarray(inputs["W_src"], np.float32)
    b_src = np.asarray(inputs["b_src"], np.float32)
    W_tgt = np.asarray(inputs["W_tgt"], np.float32)
    b_tgt = np.asarray(inputs["b_tgt"], np.float32)
    w_out = np.asarray(inputs["w_out"], np.float32)
    dense_W = np.asarray(inputs["dense_W"], np.float32)
    dense_b = np.asarray(inputs["dense_b"], np.float32)
    clf_W = np.asarray(inputs["clf_W"], np.float32)

    def tile_wT(w_block):
        # [rows, H] weight block -> rhs chunks [kc, k(128), rows]
        wT = np.ascontiguousarray(w_block.T)          # [H(k), rows]
        return np.ascontiguousarray(wT.reshape(KC, P, w_block.shape[0]))

    # prefix-Z lhsT: w_out chunk kc on cols {0,32,64,96} of its 97-block
    woutZ = np.zeros((P, KC * WZ), np.float32)
    for kc in range(KC):
        for j in range(NSUB):
            woutZ[:, kc * WZ + 32 * j] = w_out[kc * P:(kc + 1) * P]

    maps = []
    for c in range(8):
        b, r = c // 4, c % 4
        hr0 = (c % 4) * HRANGE
        rows = np.concatenate(
            [np.arange(tt * H + hr0, tt * H + hr0 + HRANGE) for tt in range(T)]
        )
        clf_slice = clf_W[0, hr0:hr0 + HRANGE]
        clfZ = np.zeros((NZC, P, T), np.float32)
        for zc in range(NZC):
            for p in range(min(P, DROWS - zc * P)):
                row = zc * P + p
                clfZ[zc, p, row // HRANGE] = clf_slice[row % HRANGE]
        maps.append({
            "hidT": np.ascontiguousarray(hidden[b].T).reshape(KC, P, L),
            "wsrcT": tile_wT(W_src[r * H:(r + 1) * H]),
            "wtgtT": tile_wT(W_tgt[r * H:(r + 1) * H]),
            "bsrc": np.ascontiguousarray(b_src[r * H:(r + 1) * H]).reshape(1, H),
            "btgt": np.ascontiguousarray(b_tgt[r * H:(r + 1) * H]).reshape(1, H),
            "woutZ": woutZ,
            "dwT": tile_wT(dense_W[rows]),
            "db": np.ascontiguousarray(dense_b[rows]).reshape(1, DROWS),
            "clfZ": clfZ,
        })
    return maps


def _assemble(results, inputs):
    clf_b = np.asarray(inputs["clf_b"], np.float32)
    rel = np.empty((B, R, L, L), np.float32)
    h_src = np.empty((B, L, R, H), np.float32)
    h_tgt = np.empty((B, L, R, H), np.float32)
    mask = np.zeros((B, T, L), np.float32)
    for c in range(8):
        b, r = c // 4, c % 4
        out = results[c]
        rel[b, r] = out["rel"].reshape(L, L)
        h_src[b, :, r, :] = out["hsrc"]
        h_tgt[b, :, r, :] = out["htgt"]
        mask[b] += out["maskp"]
    mask += clf_b[0]
    return rel, mask, h_src, h_tgt


def _run(inputs, trace=False):
    from concourse import bass_utils

    if "nc" not in _cache:
        _cache["nc"] = _build()
    res = bass_utils.run_bass_kernel_spmd(
        _cache["nc"], _in_maps(inputs), core_ids=list(range(8)), trace=trace,
    )
    return _assemble(res.results, inputs), res


def kernel(**inputs):
    out, _ = _run(inputs, trace=False)
    return out
